# revision 36
# baseline (speedup 1.0000x reference)
"""Trainium2 Bass kernel for a pre-LN transformer block (B=2, T=2048, C=1024, H=16).

Strategy (8 NeuronCores, SPMD):
  - Tensor-parallel over heads for attention: core c owns heads {2c, 2c+1}.
    Every core computes LN1 + its qkv slice for ALL 4096 tokens (both batches),
    runs causal attention for its 2 heads, producing y^T slices [128, 4096].
  - One 8-core AllToAll switches layout from head-parallel to token-parallel:
    after it, core c holds y^T[:, 512c:512c+512] (all 1024 columns, its 512 rows).
  - Row-parallel from there: proj + residual + LN2 + MLP + residual on the
    core's own 512 token rows, with full proj/fc/fc_proj weights. No all-reduce.
  - Matmul operands in bf16 (fp32 PSUM accumulation); LN statistics, softmax
    denominators and residual stream kept in fp32.

Layout conventions on chip:
  - "T" suffix = transposed activations [feature_partitions, token_free].
  - Attention computes S^T = k q^T (keys on partitions), exp on ScalarE,
    O^T via stationary [v | 1] so the softmax denominator falls out of the
    same matmul as an extra output row. Causal masking is multiplicative
    (0/1) after exp, only on the 4 block-diagonal key tiles per q-chunk.
"""

import numpy as np
import ml_dtypes

from concourse import bass, bacc, tile, mybir, bass_utils

BF16 = mybir.dt.bfloat16
F32 = mybir.dt.float32
F32R = mybir.dt.float32r
AX = mybir.AxisListType
OP = mybir.AluOpType
AF = mybir.ActivationFunctionType

B, T, C, H, HD = 2, 2048, 1024, 16, 64
NCORES = 8
BT = B * T                  # 4096 global tokens
RPC = BT // NCORES          # 512 rows per core
NCH = BT // 512             # 8 token chunks of 512
CB = C // 128               # 8 contraction blocks
HT = 4 * C // 128           # 32 hidden tiles
EPS = 1e-5

_cache = {}


def _fr(ap):
    return ap.bitcast(F32R)


def build():
    nc = bacc.Bacc("TRN2", target_bir_lowering=False, debug=False, num_devices=NCORES)

    def din(name, shape, dt=BF16):
        return nc.dram_tensor(name, list(shape), dt, kind="ExternalInput").ap()

    xT8 = din("xT8", [NCH, 128, 8 * 512])                 # x transposed, chunked
    xrows = din("xrows", [4, 128, C], F32)                # own residual rows
    wqk = din("wqk", [CB, 128, 256])                      # [q_h0|q_h1|k_h0|k_h1]
    bqk = din("bqk", [256, 1], F32)
    cqk = din("cqk", [256, 1], F32)
    wv = din("wv", [CB, 128, 128])                        # [v_h0|v_h1]
    bv = din("bv", [128, 1], F32)
    cv = din("cv", [128, 1], F32)
    wproj = din("wproj", [4, 128, 4 * 512])
    bproj = din("bproj", [1, C], F32R)
    wfc = din("wfc", [8, 128, 8 * 512])                   # htg -> [cb | 4 ht cols]
    bfc = din("bfc", [128, HT], F32)
    wfcp = din("wfcp", [HT, 128, 1024])
    bfcp = din("bfcp", [1, C], F32R)
    maskd = din("maskd", [4, 128, 512])                   # 0/1 causal diag masks
    ident = din("ident", [128, 128])
    onesr = din("onesr", [1, 128], F32R)
    out_rows = nc.dram_tensor("out_rows", [4, 128, C], F32, kind="ExternalOutput").ap()

    with tile.TileContext(nc) as tc:
        with tc.tile_pool(name="persist", bufs=1) as pp, \
             tc.tile_pool(name="work", bufs=2) as wk, \
             tc.tile_pool(name="psum", bufs=2, space="PSUM") as ps, \
             tc.tile_pool(name="dram", bufs=1, space="DRAM") as dram:

            # ---------- constants / small persistent tiles ----------
            ones_bf = pp.tile([128, 1], BF16, tag="ones_bf")
            nc.vector.memset(ones_bf[:], 1.0 / C)
            ones_row = pp.tile([1, 128], F32R, tag="ones_row")
            nc.sync.dma_start(ones_row[:], onesr[:])
            eps1 = pp.tile([1, 1], F32, tag="eps1")
            nc.vector.memset(eps1[:], EPS)
            eps128 = pp.tile([128, 1], F32, tag="eps128")
            nc.vector.memset(eps128[:], EPS)
            idn = pp.tile([128, 128], BF16, tag="idn")
            nc.sync.dma_start(idn[:], ident[:])
            msk = [pp.tile([128, 512], BF16, tag=f"msk{m}", name=f"msk{m}") for m in range(4)]
            for m in range(4):
                nc.sync.dma_start(msk[m][:], maskd[m])
            bq_sb = pp.tile([128, 1], F32, tag="bq_sb")
            bk_sb = pp.tile([128, 1], F32, tag="bk_sb")
            nc.sync.dma_start(bq_sb[:], bqk[0:128, :])
            nc.sync.dma_start(bk_sb[:], bqk[128:256, :])
            bv_sb = pp.tile([128, 1], F32, tag="bv_sb")
            nc.sync.dma_start(bv_sb[:], bv[:])
            cq_sb = pp.tile([128, 1], F32, tag="cq_sb")
            ck_sb = pp.tile([128, 1], F32, tag="ck_sb")
            nc.sync.dma_start(cq_sb[:], cqk[0:128, :])
            nc.sync.dma_start(ck_sb[:], cqk[128:256, :])
            cv_sb = pp.tile([128, 1], F32, tag="cv_sb")
            nc.sync.dma_start(cv_sb[:], cv[:])
            bproj_sb = pp.tile([1, C], F32R, tag="bproj_sb")
            nc.sync.dma_start(bproj_sb[:], bproj[:])
            bfc_sb = pp.tile([128, HT], F32, tag="bfc_sb")
            nc.sync.dma_start(bfc_sb[:], bfc[:])
            bfcp_sb = pp.tile([1, C], F32R, tag="bfcp_sb")
            nc.sync.dma_start(bfcp_sb[:], bfcp[:])
            wqk_sb = [pp.tile([128, 256], BF16, tag=f"wqk{cb}", name=f"wqk{cb}") for cb in range(CB)]
            wv_sb = [pp.tile([128, 128], BF16, tag=f"wv{cb}", name=f"wvsb{cb}") for cb in range(CB)]
            for cb in range(CB):
                nc.sync.dma_start(wqk_sb[cb][:], wqk[cb])
                nc.sync.dma_start(wv_sb[cb][:], wv[cb])

            # persistent activation tensors
            qt = [pp.tile([128, T], BF16, tag=f"qt{b}", name=f"qt{b}") for b in range(B)]
            kt_ = [pp.tile([128, T], BF16, tag=f"kt{b}", name=f"ktt{b}") for b in range(B)]
            v1 = [[pp.tile([128, 130], BF16, tag=f"v1_{b}_{i}", name=f"v1_{b}_{i}") for i in range(16)]
                  for b in range(B)]
            for b in range(B):
                for i in range(16):
                    # ones columns (65*hp + 64) used as the denominator column
                    oc = v1[b][i][:, 0:130].rearrange("p (h d) -> p h d", h=2)
                    nc.vector.memset(oc[:, :, 64:65], 1.0)

            ib = dram.tile([NCH, 128, 512], BF16, tag="ib")
            ob = dram.tile([NCH, 128, 512], BF16, tag="ob")

            # ===== Phase A: LN1 (folded post-matmul) + qkv (transposed), per chunk =====
            # qkv = ln1(x) @ W + b with ln1 folded AFTER the matmul:
            #   out[p,t] = rs_t*G[p,t] - ((mu*rs)_t*colsum(W)[p] - b[p]),  G = x @ W
            # so the normalize runs on 3 output tiles instead of 8 input tiles.
            for ch in range(NCH):
                b, jj = divmod(ch, 4)
                xta = wk.tile([128, 8 * 512], BF16, tag="xt", bufs=2, name="xta")
                nc.sync.dma_start(xta[:], xT8[ch])
                xt = [xta[:, 512 * pt:512 * (pt + 1)] for pt in range(CB)]
                # stats via PE with lhsT = 1/C: st1 = mean(x), st2 = mean(x^2)
                st1 = ps.tile([1, 512], F32, tag="st", bufs=2)
                st2 = ps.tile([1, 512], F32, tag="st", bufs=2)
                for pt in range(CB):
                    nc.tensor.matmul(st1[:], ones_bf[:], xt[pt],
                                     start=(pt == 0), stop=(pt == CB - 1))
                for h in range(2):
                    sqa = wk.tile([128, 4 * 512], BF16, tag="sq", bufs=1, name="sqa")
                    nc.vector.tensor_tensor(
                        sqa[:], xta[:, 2048 * h:2048 * (h + 1)],
                        xta[:, 2048 * h:2048 * (h + 1)], op=OP.mult)
                    for pp_i in range(4):
                        pt = 4 * h + pp_i
                        nc.tensor.matmul(st2[:], ones_bf[:],
                                         sqa[:, 512 * pp_i:512 * (pp_i + 1)],
                                         start=(pt == 0), stop=(pt == CB - 1))
                mu2 = wk.tile([1, 512], F32, tag="arow", bufs=3, name="mu2")
                nc.scalar.activation(mu2[:], st1[:], AF.Square)
                var = wk.tile([1, 512], F32, tag="arow", bufs=3, name="var")
                nc.vector.tensor_tensor(var[:], st2[:], mu2[:], op=OP.subtract)
                rs = wk.tile([1, 512], F32, tag="arow", bufs=3, name="rs")
                nc.scalar.activation(rs[:], var[:], AF.Abs_reciprocal_sqrt, bias=eps1[:])
                am = wk.tile([1, 512], F32, tag="arow", bufs=3, name="am")
                nc.vector.tensor_tensor(am[:], st1[:], rs[:], op=OP.mult)
                bc_rs = wk.tile([128, 512], F32, tag="bc", bufs=2, name="bc_rs")
                bc_a = wk.tile([128, 512], F32, tag="bc", bufs=2, name="bc_a")
                nc.gpsimd.partition_broadcast(bc_rs[:], rs[:])
                nc.gpsimd.partition_broadcast(bc_a[:], am[:])

                def fold(dst_ap, gp, cs_ap, b_ap):
                    # dst = rs*G - (bc_a*cs - b)
                    m = wk.tile([128, 512], BF16, tag="foldm", bufs=2, name="m")
                    nc.vector.tensor_scalar(m[:], bc_a, cs_ap, b_ap,
                                            op0=OP.mult, op1=OP.subtract)
                    p1 = wk.tile([128, 512], F32, tag="foldp", bufs=2, name="p1")
                    nc.vector.tensor_tensor(p1[:], gp[:], bc_rs, op=OP.mult)
                    nc.vector.tensor_tensor(dst_ap, p1[:], m[:], op=OP.subtract)

                pq = ps.tile([128, 512], F32, tag="big", bufs=3)
                for pt in range(CB):
                    nc.tensor.matmul(pq[:], wqk_sb[pt][:, 0:128], xt[pt],
                                     start=(pt == 0), stop=(pt == CB - 1))
                fold(qt[b][:, 512 * jj:512 * (jj + 1)], pq, cq_sb[:], bq_sb[:])
                pk = ps.tile([128, 512], F32, tag="big", bufs=3)
                for pt in range(CB):
                    nc.tensor.matmul(pk[:], wqk_sb[pt][:, 128:256], xt[pt],
                                     start=(pt == 0), stop=(pt == CB - 1))
                fold(kt_[b][:, 512 * jj:512 * (jj + 1)], pk, ck_sb[:], bk_sb[:])
                # v (transposed) then per-128 transpose into row-layout v1
                pv = ps.tile([128, 512], F32, tag="big", bufs=3)
                for pt in range(CB):
                    nc.tensor.matmul(pv[:], wv_sb[pt][:], xt[pt],
                                     start=(pt == 0), stop=(pt == CB - 1))
                vts = wk.tile([128, 512], BF16, tag="vts")
                fold(vts[:], pv, cv_sb[:], bv_sb[:])
                for tt in range(4):
                    ptr = ps.tile([128, 128], BF16, tag="big", bufs=3)
                    nc.tensor.transpose(ptr[:], vts[:, 128 * tt:128 * (tt + 1)], idn[:])
                    dst = v1[b][4 * jj + tt][:, 0:130].rearrange(
                        "p (h d) -> p h d", h=2)
                    nc.vector.tensor_copy(
                        dst[:, :, 0:64],
                        ptr[:].rearrange("p (h d) -> p h d", h=2),
                    )

            # =========== Phase B: causal attention, 2 heads packed ===========
            for b in range(B):
                for jj in range(4):
                    nkt = 4 * jj + 4
                    po = [ps.tile([65, 512], F32, tag="po", bufs=3, name=f"po{hp}") for hp in range(2)]
                    for kti in range(nkt):
                        for hp in range(2):
                            s = ps.tile([128, 512], F32, tag="big", bufs=3)
                            nc.tensor.matmul(
                                s[:],
                                kt_[b][64 * hp:64 * (hp + 1), 128 * kti:128 * (kti + 1)],
                                qt[b][64 * hp:64 * (hp + 1), 512 * jj:512 * (jj + 1)],
                                start=True, stop=True,
                            )
                            pt_sb = wk.tile([128, 512], BF16, tag="pt_sb", bufs=3, name="pt_sb")
                            nc.scalar.activation(pt_sb[:], s[:], AF.Exp)
                            m = kti - 4 * jj
                            if m >= 0:
                                nc.gpsimd.tensor_tensor(pt_sb[:], pt_sb[:], msk[m][:],
                                                        op=OP.mult)
                            nc.tensor.matmul(
                                po[hp][:],
                                v1[b][kti][:, 65 * hp:65 * hp + 65],
                                pt_sb[:],
                                start=(kti == 0), stop=(kti == nkt - 1),
                            )
                    for hp in range(2):
                        recip = wk.tile([1, 512], F32, tag="recip", bufs=1)
                        rsc = wk.tile([1, 512], F32, tag="rsc", bufs=1, name="rsc")
                        nc.vector.tensor_copy(rsc[:], po[hp][64:65, :])
                        nc.vector.reciprocal_approx_fast(recip[:], rsc[:])
                        bcp = wk.tile([64, 512], F32, tag="bcb", bufs=1, name="bcp")
                        nc.gpsimd.partition_broadcast(bcp[:], recip[:])
                        osb = wk.tile([64, 512], BF16, tag="osb")
                        nc.scalar.activation(osb[:], po[hp][0:64, :], AF.Copy)
                        yt = wk.tile([64, 512], BF16, tag="yt")
                        nc.vector.tensor_tensor(yt[:], osb[:], bcp[:], op=OP.mult)
                        nc.sync.dma_start(ib[4 * b + jj, 64 * hp:64 * (hp + 1), :], yt[:])

            # =========== Phase C: AllToAll (head-parallel -> token-parallel) ===========
            # prefetch everything the A2A-dependent phases need BEFORE the
            # sync queue blocks on the collective semaphore; x2 doubles as
            # the residual accumulator (initialized from x rows by DMA)
            x2 = [pp.tile([128, C], F32, tag=f"x2{tt}", name=f"x2{tt}") for tt in range(4)]
            for tt in range(4):
                nc.sync.dma_start(x2[tt][:], xrows[tt])
            wpj = [wk.tile([128, 4 * 512], BF16, tag="wpj", bufs=2, name=f"wpj{q}")
                   for q in range(4)]
            for q in range(2):
                nc.sync.dma_start(wpj[q][:], wproj[q])
            nc.gpsimd.collective_compute(
                "AllToAll", OP.bypass,
                ins=[ib.opt()], outs=[ob.opt()],
                replica_groups=[list(range(NCORES))],
            )
            yT = [pp.tile([128, 512], BF16, tag=f"yT{cb}", name=f"yT{cb}") for cb in range(CB)]
            for cb in range(CB):
                nc.scalar.dma_start(yT[cb][:], ob[cb])

            # =========== Phase D: proj + residual (row-parallel, own 512 rows) ==========
            # quarter q of wproj = (nh, cb-half); nh-outer so each quarter is
            # loaded once; psum per tt accumulates across both quarters
            for nh in range(2):
                pps = [ps.tile([128, 512], F32, tag="big", bufs=3, name=f"pps{tt}")
                       for tt in range(4)]
                for cbh in range(2):
                    q = 2 * nh + cbh
                    if q >= 2:
                        nc.sync.dma_start(wpj[q][:], wproj[q])
                    for tt in range(4):
                        for cbl in range(4):
                            cb = 4 * cbh + cbl
                            nc.tensor.matmul(pps[tt][:],
                                             yT[cb][:, 128 * tt:128 * (tt + 1)],
                                             wpj[q][:, 512 * cbl:512 * (cbl + 1)],
                                             start=(cb == 0), stop=False)
                for tt in range(4):
                    nc.tensor.matmul(pps[tt][:], ones_row[0:1, :],
                                     bproj_sb[0:1, 512 * nh:512 * (nh + 1)],
                                     start=False, stop=True)
                    nc.vector.tensor_tensor(
                        x2[tt][:, 512 * nh:512 * (nh + 1)], pps[tt][:],
                        x2[tt][:, 512 * nh:512 * (nh + 1)], op=OP.add)

            # =========== Phase E: LN2 + transpose ===========
            ln2T = [pp.tile([128, 512], BF16, tag=f"ln2T{cb}", name=f"ln2T{cb}") for cb in range(CB)]
            for tt in range(4):
                s1 = wk.tile([128, 1], F32, tag="e_s1")
                nc.vector.reduce_sum(s1[:], x2[tt][:], axis=AX.X)
                nmu = wk.tile([128, 1], F32, tag="e_nmu")
                nc.vector.tensor_scalar(nmu[:], s1[:], -1.0 / C, None, op0=OP.mult)
                sqs = wk.tile([128, C], F32, tag="sq", bufs=1, name="sqs")
                s2 = wk.tile([128, 1], F32, tag="e_s2")
                nc.scalar.activation(sqs[:], x2[tt][:], AF.Square, accum_out=s2[:])
                m2 = wk.tile([128, 1], F32, tag="e_m2")
                nc.vector.tensor_tensor(m2[:], nmu[:], nmu[:], op=OP.mult)
                var = wk.tile([128, 1], F32, tag="e_var")
                nc.vector.tensor_scalar(var[:], s2[:], 1.0 / C, None, op0=OP.mult)
                nc.vector.tensor_tensor(var[:], var[:], m2[:], op=OP.subtract)
                sd = wk.tile([128, 1], F32, tag="e_sd")
                nc.scalar.activation(sd[:], var[:], AF.Sqrt, bias=eps128[:])
                rs2 = wk.tile([128, 1], F32, tag="e_rs2")
                nc.vector.reciprocal(rs2[:], sd[:])
                na = wk.tile([128, 1], F32, tag="e_na")
                nc.vector.tensor_tensor(na[:], nmu[:], rs2[:], op=OP.mult)
                lr = wk.tile([128, C], BF16, tag="e_lr", bufs=1)
                nc.scalar.activation(lr[:], x2[tt][:], AF.Identity,
                                     bias=na[:], scale=rs2[:])
                for cb in range(CB):
                    ptr = ps.tile([128, 128], BF16, tag="big", bufs=3)
                    nc.tensor.transpose(ptr[:], lr[:, 128 * cb:128 * (cb + 1)], idn[:])
                    nc.vector.tensor_copy(ln2T[cb][:, 128 * tt:128 * (tt + 1)], ptr[:])

            # =========== Phase F: fc + gelu (transposed hidden) ===========
            ghT = [pp.tile([128, 512], BF16, tag=f"ghT{ht}", name=f"ghT{ht}") for ht in range(HT)]
            for htg in range(8):
                wt = wk.tile([128, 8 * 512], BF16, tag="wfc", bufs=2, name="wt")
                nc.sync.dma_start(wt[:], wfc[htg])
                for sub in range(4):
                    ht = 4 * htg + sub
                    ph = ps.tile([128, 512], F32, tag="big", bufs=3)
                    for cb in range(CB):
                        nc.tensor.matmul(
                            ph[:], wt[:, 512 * cb + 128 * sub:512 * cb + 128 * (sub + 1)],
                            ln2T[cb][:], start=(cb == 0), stop=(cb == CB - 1))
                    nc.scalar.activation(ghT[ht][:], ph[:], AF.Gelu,
                                         bias=bfc_sb[:, ht:ht + 1])

            # =========== Phase G: fc_proj + residual -> output rows ===========
            for half in range(2):
                pg = {}
                for tt in range(2 * half, 2 * half + 2):
                    for nh in range(2):
                        pg[(tt, nh)] = ps.tile([128, 512], F32, name=f"pg{tt}{nh}",
                                               tag=("st" if nh == 0 else "po"),
                                               bufs=(2 if nh == 0 else 3))
                for ht in range(HT):
                    w = wk.tile([128, 1024], BF16, tag="wfcp", bufs=4, name="wfcp")
                    nc.scalar.dma_start(w[:], wfcp[ht])
                    for nh in range(2):
                        for tt in range(2 * half, 2 * half + 2):
                            nc.tensor.matmul(pg[(tt, nh)][:],
                                             ghT[ht][:, 128 * tt:128 * (tt + 1)],
                                             w[:, 512 * nh:512 * (nh + 1)],
                                             start=(ht == 0), stop=False)
                for tt in range(2 * half, 2 * half + 2):
                    orow = wk.tile([128, C], F32, tag="orow")
                    for nh in range(2):
                        nc.tensor.matmul(pg[(tt, nh)][:], ones_row[0:1, :],
                                         bfcp_sb[0:1, 512 * nh:512 * (nh + 1)],
                                         start=False, stop=True)
                        nc.vector.tensor_tensor(
                            orow[:, 512 * nh:512 * (nh + 1)], pg[(tt, nh)][:],
                            x2[tt][:, 512 * nh:512 * (nh + 1)], op=OP.add)
                    nc.sync.dma_start(out_rows[tt], orow[:])

    nc.compile()
    return nc


def _prep(inputs):
    """Host-side sharding/layout prep. Returns in_maps for the 8 cores."""
    f32 = np.float32
    bf = ml_dtypes.bfloat16
    x = np.asarray(inputs["x"], f32)
    ln1_w = np.asarray(inputs["ln1_w"], f32)
    ln1_b = np.asarray(inputs["ln1_b"], f32)
    attn_w = np.asarray(inputs["attn_w"], f32)
    attn_b = np.asarray(inputs["attn_b"], f32)
    proj_w = np.asarray(inputs["proj_w"], f32)
    proj_b = np.asarray(inputs["proj_b"], f32)
    ln2_w = np.asarray(inputs["ln2_w"], f32)
    ln2_b = np.asarray(inputs["ln2_b"], f32)
    fc_w = np.asarray(inputs["fc_w"], f32)
    fc_b = np.asarray(inputs["fc_b"], f32)
    fc_proj_w = np.asarray(inputs["fc_proj_w"], f32)
    fc_proj_b = np.asarray(inputs["fc_proj_b"], f32)

    # fold LN affine params into the following matmuls (exact linear identities)
    aw = ln1_w[:, None] * attn_w
    ab = ln1_b @ attn_w + attn_b
    fw = ln2_w[:, None] * fc_w
    fb = ln2_b @ fc_w + fc_b

    sc = 1.0 / np.sqrt(HD)
    xg = x.reshape(BT, C)                                  # global token rows
    # xT8[ch, p, 512*pt + q] = x_g[512*ch + q, 128*pt + p]
    xT8 = np.ascontiguousarray(
        xg.reshape(NCH, 512, CB, 128).transpose(0, 3, 2, 1).reshape(NCH, 128, 8 * 512)
    ).astype(bf)
    # wproj[2*nh+cbh][p, 512*cbl + j] = proj_w[128*(4*cbh+cbl) + p, 512*nh + j]
    wproj_h = np.ascontiguousarray(
        proj_w.reshape(2, 4, 128, 2, 512).transpose(3, 0, 2, 1, 4)
        .reshape(4, 128, 4 * 512)).astype(bf)
    # wfc[htg][p, 512*cb + 128*sub + j] = fw[128*cb + p, 512*htg + 128*sub + j]
    wfc_h = np.ascontiguousarray(
        fw.reshape(CB, 128, 8, 512).transpose(2, 1, 0, 3).reshape(8, 128, 8 * 512)
    ).astype(bf)
    bfc_h = np.ascontiguousarray(fb.reshape(HT, 128).T).astype(f32)
    wfcp_h = np.ascontiguousarray(fc_proj_w.reshape(HT, 128, 1024)).astype(bf)

    maskd = np.zeros((4, 128, 512), np.float32)
    for m in range(4):
        maskd[m] = (128 * m + np.arange(128)[:, None]) <= np.arange(512)[None, :]
    maskd = maskd.astype(bf)
    ident = np.eye(128, dtype=np.float32).astype(bf)

    shared = dict(
        xT8=xT8,
        wproj=wproj_h, bproj=proj_b.reshape(1, C).astype(f32),
        wfc=wfc_h, bfc=bfc_h,
        wfcp=wfcp_h, bfcp=fc_proj_b.reshape(1, C).astype(f32),
        maskd=maskd, ident=ident, onesr=np.ones((1, 128), f32),
    )

    in_maps = []
    for c in range(NCORES):
        h0 = 2 * c
        qcols = aw[:, 64 * h0:64 * h0 + 128] * sc          # [1024, 128] both heads' q
        kcols = aw[:, C + 64 * h0:C + 64 * h0 + 128]
        vcols = aw[:, 2 * C + 64 * h0:2 * C + 64 * h0 + 128]
        wqk_full = np.concatenate([qcols, kcols], axis=1).astype(bf)
        wqk_c = wqk_full.reshape(CB, 128, 256)
        bq = ab[64 * h0:64 * h0 + 128] * sc
        bk = ab[C + 64 * h0:C + 64 * h0 + 128]
        bqk_c = np.concatenate([bq, bk]).reshape(256, 1).astype(f32)
        cqk_c = wqk_full.astype(f32).sum(axis=0).reshape(256, 1).astype(f32)
        wv_full = vcols.astype(bf)
        wv_c = wv_full.reshape(CB, 128, 128)
        bv_c = ab[2 * C + 64 * h0:2 * C + 64 * h0 + 128].reshape(128, 1).astype(f32)
        cv_c = wv_full.astype(f32).sum(axis=0).reshape(128, 1).astype(f32)
        xrows_c = xg[RPC * c:RPC * (c + 1)].reshape(4, 128, C).astype(f32)
        m = dict(shared)
        m.update(wqk=wqk_c, bqk=bqk_c, cqk=cqk_c, wv=wv_c, bv=bv_c, cv=cv_c,
                 xrows=xrows_c)
        in_maps.append(m)
    return in_maps


def kernel(**inputs) -> np.ndarray:
    if "nc" not in _cache:
        _cache["nc"] = build()
    nc = _cache["nc"]
    in_maps = _prep(inputs)
    res = bass_utils.run_bass_kernel_spmd(nc, in_maps, core_ids=list(range(NCORES)))
    out = np.concatenate(
        [res.results[c]["out_rows"].reshape(RPC, C) for c in range(NCORES)], axis=0)
    return out.reshape(B, T, C).astype(np.float32)


# revision 39
# speedup vs baseline: 1.0213x; 1.0213x over previous
"""Trainium2 Bass kernel for a pre-LN transformer block (B=2, T=2048, C=1024, H=16).

Strategy (8 NeuronCores, SPMD):
  - Tensor-parallel over heads for attention: core c owns heads {2c, 2c+1}.
    Every core computes LN1 + its qkv slice for ALL 4096 tokens (both batches),
    runs causal attention for its 2 heads, producing y^T slices [128, 4096].
  - One 8-core AllToAll switches layout from head-parallel to token-parallel:
    after it, core c holds y^T[:, 512c:512c+512] (all 1024 columns, its 512 rows).
  - Row-parallel from there: proj + residual + LN2 + MLP + residual on the
    core's own 512 token rows, with full proj/fc/fc_proj weights. No all-reduce.
  - Matmul operands in bf16 (fp32 PSUM accumulation); LN statistics, softmax
    denominators and residual stream kept in fp32.

Layout conventions on chip:
  - "T" suffix = transposed activations [feature_partitions, token_free].
  - Attention computes S^T = k q^T (keys on partitions), exp on ScalarE,
    O^T via stationary [v | 1] so the softmax denominator falls out of the
    same matmul as an extra output row. Causal masking is multiplicative
    (0/1) after exp, only on the 4 block-diagonal key tiles per q-chunk.
"""

import numpy as np
import ml_dtypes

from concourse import bass, bacc, tile, mybir, bass_utils

BF16 = mybir.dt.bfloat16
F32 = mybir.dt.float32
F32R = mybir.dt.float32r
AX = mybir.AxisListType
OP = mybir.AluOpType
AF = mybir.ActivationFunctionType

B, T, C, H, HD = 2, 2048, 1024, 16, 64
NCORES = 8
BT = B * T                  # 4096 global tokens
RPC = BT // NCORES          # 512 rows per core
NCH = BT // 512             # 8 token chunks of 512
CB = C // 128               # 8 contraction blocks
HT = 4 * C // 128           # 32 hidden tiles
EPS = 1e-5

_cache = {}


def _fr(ap):
    return ap.bitcast(F32R)


def build():
    nc = bacc.Bacc("TRN2", target_bir_lowering=False, debug=False, num_devices=NCORES)

    def din(name, shape, dt=BF16):
        return nc.dram_tensor(name, list(shape), dt, kind="ExternalInput").ap()

    xT8 = din("xT8", [NCH, 128, 8 * 512])                 # x transposed, chunked
    xrows = din("xrows", [4, 128, C], F32)                # own residual rows
    wqk = din("wqk", [CB, 128, 256])                      # [q_h0|q_h1|k_h0|k_h1]
    bqk = din("bqk", [256, 1], F32)
    cqk = din("cqk", [256, 1], F32)
    wv = din("wv", [CB, 128, 128])                        # [v_h0|v_h1]
    bv = din("bv", [128, 1], F32)
    cv = din("cv", [128, 1], F32)
    wproj = din("wproj", [4, 128, 4 * 512])
    bproj = din("bproj", [1, C], F32R)
    wfc = din("wfc", [8, 128, 8 * 512])                   # htg -> [cb | 4 ht cols]
    bfc = din("bfc", [128, HT], F32)
    wfcp = din("wfcp", [HT, 128, 1024])
    bfcp = din("bfcp", [1, C], F32R)
    maskd = din("maskd", [4, 128, 512])                   # 0/1 causal diag masks
    ident = din("ident", [128, 128])
    onesr = din("onesr", [1, 128], F32R)
    out_rows = nc.dram_tensor("out_rows", [4, 128, C], F32, kind="ExternalOutput").ap()

    with tile.TileContext(nc) as tc:
        with tc.tile_pool(name="persist", bufs=1) as pp, \
             tc.tile_pool(name="work", bufs=2) as wk, \
             tc.tile_pool(name="psum", bufs=2, space="PSUM") as ps, \
             tc.tile_pool(name="dram", bufs=1, space="DRAM") as dram:

            # ---------- constants / small persistent tiles ----------
            ones_bf = pp.tile([128, 1], BF16, tag="ones_bf")
            nc.vector.memset(ones_bf[:], 1.0 / C)
            ones_row = pp.tile([1, 128], F32R, tag="ones_row")
            nc.sync.dma_start(ones_row[:], onesr[:])
            eps1 = pp.tile([1, 1], F32, tag="eps1")
            nc.vector.memset(eps1[:], EPS)
            eps128 = pp.tile([128, 1], F32, tag="eps128")
            nc.vector.memset(eps128[:], EPS)
            idn = pp.tile([128, 128], BF16, tag="idn")
            nc.sync.dma_start(idn[:], ident[:])
            msk = [pp.tile([128, 512], BF16, tag=f"msk{m}", name=f"msk{m}") for m in range(4)]
            for m in range(4):
                nc.sync.dma_start(msk[m][:], maskd[m])
            bq_sb = pp.tile([128, 1], F32, tag="bq_sb")
            bk_sb = pp.tile([128, 1], F32, tag="bk_sb")
            nc.sync.dma_start(bq_sb[:], bqk[0:128, :])
            nc.sync.dma_start(bk_sb[:], bqk[128:256, :])
            bv_sb = pp.tile([128, 1], F32, tag="bv_sb")
            nc.sync.dma_start(bv_sb[:], bv[:])
            cq_sb = pp.tile([128, 1], F32, tag="cq_sb")
            ck_sb = pp.tile([128, 1], F32, tag="ck_sb")
            nc.sync.dma_start(cq_sb[:], cqk[0:128, :])
            nc.sync.dma_start(ck_sb[:], cqk[128:256, :])
            cv_sb = pp.tile([128, 1], F32, tag="cv_sb")
            nc.sync.dma_start(cv_sb[:], cv[:])
            bproj_sb = pp.tile([1, C], F32R, tag="bproj_sb")
            nc.sync.dma_start(bproj_sb[:], bproj[:])
            bfc_sb = pp.tile([128, HT], F32, tag="bfc_sb")
            nc.sync.dma_start(bfc_sb[:], bfc[:])
            bfcp_sb = pp.tile([1, C], F32R, tag="bfcp_sb")
            nc.sync.dma_start(bfcp_sb[:], bfcp[:])
            wqk_sb = [pp.tile([128, 256], BF16, tag=f"wqk{cb}", name=f"wqk{cb}") for cb in range(CB)]
            wv_sb = [pp.tile([128, 128], BF16, tag=f"wv{cb}", name=f"wvsb{cb}") for cb in range(CB)]
            for cb in range(CB):
                nc.sync.dma_start(wqk_sb[cb][:], wqk[cb])
                nc.sync.dma_start(wv_sb[cb][:], wv[cb])

            # persistent activation tensors
            qt = [pp.tile([128, T], BF16, tag=f"qt{b}", name=f"qt{b}") for b in range(B)]
            kt_ = [pp.tile([128, T], BF16, tag=f"kt{b}", name=f"ktt{b}") for b in range(B)]
            v1 = [[pp.tile([128, 130], BF16, tag=f"v1_{b}_{i}", name=f"v1_{b}_{i}") for i in range(16)]
                  for b in range(B)]
            for b in range(B):
                for i in range(16):
                    # ones columns (65*hp + 64) used as the denominator column
                    oc = v1[b][i][:, 0:130].rearrange("p (h d) -> p h d", h=2)
                    nc.vector.memset(oc[:, :, 64:65], 1.0)

            ib = dram.tile([NCH, 128, 512], BF16, tag="ib")
            ob = dram.tile([NCH, 128, 512], BF16, tag="ob")

            # ===== Phase A: LN1 (folded post-matmul) + qkv (transposed), per chunk =====
            # qkv = ln1(x) @ W + b with ln1 folded AFTER the matmul:
            #   out[p,t] = rs_t*G[p,t] - ((mu*rs)_t*colsum(W)[p] - b[p]),  G = x @ W
            # so the normalize runs on 3 output tiles instead of 8 input tiles.
            for ch in range(NCH):
                b, jj = divmod(ch, 4)
                xta = wk.tile([128, 8 * 512], BF16, tag="xt", bufs=2, name="xta")
                nc.sync.dma_start(xta[:], xT8[ch])
                xt = [xta[:, 512 * pt:512 * (pt + 1)] for pt in range(CB)]
                # stats via PE with lhsT = 1/C: st1 = mean(x), st2 = mean(x^2)
                st1 = ps.tile([1, 512], F32, tag="st", bufs=2)
                st2 = ps.tile([1, 512], F32, tag="st", bufs=2)
                for pt in range(CB):
                    nc.tensor.matmul(st1[:], ones_bf[:], xt[pt],
                                     start=(pt == 0), stop=(pt == CB - 1))
                for h in range(2):
                    sqa = wk.tile([128, 4 * 512], BF16, tag="sq", bufs=1, name="sqa")
                    nc.vector.tensor_tensor(
                        sqa[:], xta[:, 2048 * h:2048 * (h + 1)],
                        xta[:, 2048 * h:2048 * (h + 1)], op=OP.mult)
                    for pp_i in range(4):
                        pt = 4 * h + pp_i
                        nc.tensor.matmul(st2[:], ones_bf[:],
                                         sqa[:, 512 * pp_i:512 * (pp_i + 1)],
                                         start=(pt == 0), stop=(pt == CB - 1))
                mu2 = wk.tile([1, 512], F32, tag="arow", bufs=3, name="mu2")
                nc.scalar.activation(mu2[:], st1[:], AF.Square)
                var = wk.tile([1, 512], F32, tag="arow", bufs=3, name="var")
                nc.vector.tensor_tensor(var[:], st2[:], mu2[:], op=OP.subtract)
                rs = wk.tile([1, 512], F32, tag="arow", bufs=3, name="rs")
                nc.scalar.activation(rs[:], var[:], AF.Abs_reciprocal_sqrt, bias=eps1[:])
                am = wk.tile([1, 512], F32, tag="arow", bufs=3, name="am")
                nc.vector.tensor_tensor(am[:], st1[:], rs[:], op=OP.mult)
                bc_rs = wk.tile([128, 512], F32, tag="bc", bufs=2, name="bc_rs")
                bc_a = wk.tile([128, 512], F32, tag="bc", bufs=2, name="bc_a")
                nc.gpsimd.partition_broadcast(bc_rs[:], rs[:])
                nc.gpsimd.partition_broadcast(bc_a[:], am[:])

                def fold(dst_ap, gp, cs_ap, b_ap):
                    # dst = rs*G - (bc_a*cs - b)
                    m = wk.tile([128, 512], BF16, tag="foldm", bufs=2, name="m")
                    nc.vector.tensor_scalar(m[:], bc_a, cs_ap, b_ap,
                                            op0=OP.mult, op1=OP.subtract)
                    p1 = wk.tile([128, 512], F32, tag="foldp", bufs=2, name="p1")
                    nc.vector.tensor_tensor(p1[:], gp[:], bc_rs, op=OP.mult)
                    nc.vector.tensor_tensor(dst_ap, p1[:], m[:], op=OP.subtract)

                pq = ps.tile([128, 512], F32, tag="big", bufs=4)
                for pt in range(CB):
                    nc.tensor.matmul(pq[:], wqk_sb[pt][:, 0:128], xt[pt],
                                     start=(pt == 0), stop=(pt == CB - 1))
                fold(qt[b][:, 512 * jj:512 * (jj + 1)], pq, cq_sb[:], bq_sb[:])
                pk = ps.tile([128, 512], F32, tag="big", bufs=4)
                for pt in range(CB):
                    nc.tensor.matmul(pk[:], wqk_sb[pt][:, 128:256], xt[pt],
                                     start=(pt == 0), stop=(pt == CB - 1))
                fold(kt_[b][:, 512 * jj:512 * (jj + 1)], pk, ck_sb[:], bk_sb[:])
                # v (transposed) then per-128 transpose into row-layout v1
                pv = ps.tile([128, 512], F32, tag="big", bufs=4)
                for pt in range(CB):
                    nc.tensor.matmul(pv[:], wv_sb[pt][:], xt[pt],
                                     start=(pt == 0), stop=(pt == CB - 1))
                vts = wk.tile([128, 512], BF16, tag="vts", bufs=1)
                fold(vts[:], pv, cv_sb[:], bv_sb[:])
                for tt in range(4):
                    ptr = ps.tile([128, 128], BF16, tag="big", bufs=4)
                    nc.tensor.transpose(ptr[:], vts[:, 128 * tt:128 * (tt + 1)], idn[:])
                    dst = v1[b][4 * jj + tt][:, 0:130].rearrange(
                        "p (h d) -> p h d", h=2)
                    nc.vector.tensor_copy(
                        dst[:, :, 0:64],
                        ptr[:].rearrange("p (h d) -> p h d", h=2),
                    )

            # =========== Phase B: causal attention, 2 heads packed ===========
            for b in range(B):
                for jj in range(4):
                    nkt = 4 * jj + 4
                    po = [ps.tile([65, 512], F32, tag="po", bufs=2, name=f"po{hp}") for hp in range(2)]
                    for kti in range(nkt):
                        for hp in range(2):
                            s = ps.tile([128, 512], F32, tag="big", bufs=4)
                            nc.tensor.matmul(
                                s[:],
                                kt_[b][64 * hp:64 * (hp + 1), 128 * kti:128 * (kti + 1)],
                                qt[b][64 * hp:64 * (hp + 1), 512 * jj:512 * (jj + 1)],
                                start=True, stop=True,
                            )
                            pt_sb = wk.tile([128, 512], BF16, tag="pt_sb", bufs=3, name="pt_sb")
                            nc.scalar.activation(pt_sb[:], s[:], AF.Exp)
                            m = kti - 4 * jj
                            if m >= 0:
                                nc.gpsimd.tensor_tensor(pt_sb[:], pt_sb[:], msk[m][:],
                                                        op=OP.mult)
                            nc.tensor.matmul(
                                po[hp][:],
                                v1[b][kti][:, 65 * hp:65 * hp + 65],
                                pt_sb[:],
                                start=(kti == 0), stop=(kti == nkt - 1),
                            )
                    for hp in range(2):
                        # single fast eviction frees the PSUM accumulator early
                        osb = wk.tile([65, 512], F32, tag="osb", bufs=2)
                        nc.scalar.activation(osb[:], po[hp][:], AF.Copy)
                        rsc = wk.tile([1, 512], F32, tag="rsc", bufs=1, name="rsc")
                        nc.vector.tensor_copy(rsc[:], osb[64:65, :])
                        recip = wk.tile([1, 512], F32, tag="recip", bufs=1)
                        nc.vector.reciprocal_approx_fast(recip[:], rsc[:])
                        bcp = wk.tile([64, 512], F32, tag="bcb", bufs=2, name="bcp")
                        nc.gpsimd.partition_broadcast(bcp[:], recip[:])
                        yt = wk.tile([64, 512], BF16, tag="yt")
                        nc.vector.tensor_tensor(yt[:], osb[0:64, :], bcp[:], op=OP.mult)
                        nc.sync.dma_start(ib[4 * b + jj, 64 * hp:64 * (hp + 1), :], yt[:])

            # =========== Phase C: AllToAll (head-parallel -> token-parallel) ===========
            # prefetch everything the A2A-dependent phases need BEFORE the
            # sync queue blocks on the collective semaphore; x2 doubles as
            # the residual accumulator (initialized from x rows by DMA)
            x2 = [pp.tile([128, C], F32, tag=f"x2{tt}", name=f"x2{tt}") for tt in range(4)]
            for tt in range(4):
                nc.sync.dma_start(x2[tt][:], xrows[tt])
            wpj = [wk.tile([128, 4 * 512], BF16, tag="wpj", bufs=2, name=f"wpj{q}")
                   for q in range(4)]
            for q in range(2):
                nc.sync.dma_start(wpj[q][:], wproj[q])
            nc.gpsimd.collective_compute(
                "AllToAll", OP.bypass,
                ins=[ib.opt()], outs=[ob.opt()],
                replica_groups=[list(range(NCORES))],
            )
            yT = [pp.tile([128, 512], BF16, tag=f"yT{cb}", name=f"yT{cb}") for cb in range(CB)]
            for cb in range(CB):
                nc.scalar.dma_start(yT[cb][:], ob[cb])

            # =========== Phase D: proj + residual (row-parallel, own 512 rows) ==========
            # quarter q of wproj = (nh, cb-half); nh-outer so each quarter is
            # loaded once; psum per tt accumulates across both quarters
            for nh in range(2):
                pps = [ps.tile([128, 512], F32, tag="big", bufs=4, name=f"pps{tt}")
                       for tt in range(4)]
                for cbh in range(2):
                    q = 2 * nh + cbh
                    if q >= 2:
                        nc.sync.dma_start(wpj[q][:], wproj[q])
                    for tt in range(4):
                        for cbl in range(4):
                            cb = 4 * cbh + cbl
                            nc.tensor.matmul(pps[tt][:],
                                             yT[cb][:, 128 * tt:128 * (tt + 1)],
                                             wpj[q][:, 512 * cbl:512 * (cbl + 1)],
                                             start=(cb == 0), stop=False)
                for tt in range(4):
                    nc.tensor.matmul(pps[tt][:], ones_row[0:1, :],
                                     bproj_sb[0:1, 512 * nh:512 * (nh + 1)],
                                     start=False, stop=True)
                    nc.vector.tensor_tensor(
                        x2[tt][:, 512 * nh:512 * (nh + 1)], pps[tt][:],
                        x2[tt][:, 512 * nh:512 * (nh + 1)], op=OP.add)

            # =========== Phase E: LN2 + transpose ===========
            ln2T = [pp.tile([128, 512], BF16, tag=f"ln2T{cb}", name=f"ln2T{cb}") for cb in range(CB)]
            for tt in range(4):
                s1 = wk.tile([128, 1], F32, tag="e_s1")
                nc.vector.reduce_sum(s1[:], x2[tt][:], axis=AX.X)
                nmu = wk.tile([128, 1], F32, tag="e_nmu")
                nc.vector.tensor_scalar(nmu[:], s1[:], -1.0 / C, None, op0=OP.mult)
                sqs = wk.tile([128, C], F32, tag="sq", bufs=1, name="sqs")
                s2 = wk.tile([128, 1], F32, tag="e_s2")
                nc.scalar.activation(sqs[:], x2[tt][:], AF.Square, accum_out=s2[:])
                m2 = wk.tile([128, 1], F32, tag="e_m2")
                nc.vector.tensor_tensor(m2[:], nmu[:], nmu[:], op=OP.mult)
                var = wk.tile([128, 1], F32, tag="e_var")
                nc.vector.tensor_scalar(var[:], s2[:], 1.0 / C, None, op0=OP.mult)
                nc.vector.tensor_tensor(var[:], var[:], m2[:], op=OP.subtract)
                sd = wk.tile([128, 1], F32, tag="e_sd")
                nc.scalar.activation(sd[:], var[:], AF.Sqrt, bias=eps128[:])
                rs2 = wk.tile([128, 1], F32, tag="e_rs2")
                nc.vector.reciprocal(rs2[:], sd[:])
                na = wk.tile([128, 1], F32, tag="e_na")
                nc.vector.tensor_tensor(na[:], nmu[:], rs2[:], op=OP.mult)
                lr = wk.tile([128, C], BF16, tag="e_lr", bufs=1)
                nc.scalar.activation(lr[:], x2[tt][:], AF.Identity,
                                     bias=na[:], scale=rs2[:])
                for cb in range(CB):
                    ptr = ps.tile([128, 128], BF16, tag="big", bufs=4)
                    nc.tensor.transpose(ptr[:], lr[:, 128 * cb:128 * (cb + 1)], idn[:])
                    nc.vector.tensor_copy(ln2T[cb][:, 128 * tt:128 * (tt + 1)], ptr[:])

            # =========== Phase F: fc + gelu (transposed hidden) ===========
            ghT = [pp.tile([128, 512], BF16, tag=f"ghT{ht}", name=f"ghT{ht}") for ht in range(HT)]
            for htg in range(8):
                wt = wk.tile([128, 8 * 512], BF16, tag="wfc", bufs=2, name="wt")
                nc.sync.dma_start(wt[:], wfc[htg])
                for sub in range(4):
                    ht = 4 * htg + sub
                    ph = ps.tile([128, 512], F32, tag="big", bufs=4)
                    for cb in range(CB):
                        nc.tensor.matmul(
                            ph[:], wt[:, 512 * cb + 128 * sub:512 * cb + 128 * (sub + 1)],
                            ln2T[cb][:], start=(cb == 0), stop=(cb == CB - 1))
                    nc.scalar.activation(ghT[ht][:], ph[:], AF.Gelu,
                                         bias=bfc_sb[:, ht:ht + 1])

            # =========== Phase G: fc_proj + residual -> output rows ===========
            for half in range(2):
                pg = {}
                for tt in range(2 * half, 2 * half + 2):
                    for nh in range(2):
                        pg[(tt, nh)] = ps.tile([128, 512], F32, name=f"pg{tt}{nh}",
                                               tag=("st" if nh == 0 else "po"), bufs=2)
                for ht in range(HT):
                    w = wk.tile([128, 1024], BF16, tag="wfcp", bufs=4, name="wfcp")
                    nc.scalar.dma_start(w[:], wfcp[ht])
                    for nh in range(2):
                        for tt in range(2 * half, 2 * half + 2):
                            nc.tensor.matmul(pg[(tt, nh)][:],
                                             ghT[ht][:, 128 * tt:128 * (tt + 1)],
                                             w[:, 512 * nh:512 * (nh + 1)],
                                             start=(ht == 0), stop=False)
                for tt in range(2 * half, 2 * half + 2):
                    orow = wk.tile([128, C], F32, tag="orow")
                    for nh in range(2):
                        nc.tensor.matmul(pg[(tt, nh)][:], ones_row[0:1, :],
                                         bfcp_sb[0:1, 512 * nh:512 * (nh + 1)],
                                         start=False, stop=True)
                        nc.vector.tensor_tensor(
                            orow[:, 512 * nh:512 * (nh + 1)], pg[(tt, nh)][:],
                            x2[tt][:, 512 * nh:512 * (nh + 1)], op=OP.add)
                    nc.sync.dma_start(out_rows[tt], orow[:])

    nc.compile()
    return nc


def _prep(inputs):
    """Host-side sharding/layout prep. Returns in_maps for the 8 cores."""
    f32 = np.float32
    bf = ml_dtypes.bfloat16
    x = np.asarray(inputs["x"], f32)
    ln1_w = np.asarray(inputs["ln1_w"], f32)
    ln1_b = np.asarray(inputs["ln1_b"], f32)
    attn_w = np.asarray(inputs["attn_w"], f32)
    attn_b = np.asarray(inputs["attn_b"], f32)
    proj_w = np.asarray(inputs["proj_w"], f32)
    proj_b = np.asarray(inputs["proj_b"], f32)
    ln2_w = np.asarray(inputs["ln2_w"], f32)
    ln2_b = np.asarray(inputs["ln2_b"], f32)
    fc_w = np.asarray(inputs["fc_w"], f32)
    fc_b = np.asarray(inputs["fc_b"], f32)
    fc_proj_w = np.asarray(inputs["fc_proj_w"], f32)
    fc_proj_b = np.asarray(inputs["fc_proj_b"], f32)

    # fold LN affine params into the following matmuls (exact linear identities)
    aw = ln1_w[:, None] * attn_w
    ab = ln1_b @ attn_w + attn_b
    fw = ln2_w[:, None] * fc_w
    fb = ln2_b @ fc_w + fc_b

    sc = 1.0 / np.sqrt(HD)
    xg = x.reshape(BT, C)                                  # global token rows
    # xT8[ch, p, 512*pt + q] = x_g[512*ch + q, 128*pt + p]
    xT8 = np.ascontiguousarray(
        xg.reshape(NCH, 512, CB, 128).transpose(0, 3, 2, 1).reshape(NCH, 128, 8 * 512)
    ).astype(bf)
    # wproj[2*nh+cbh][p, 512*cbl + j] = proj_w[128*(4*cbh+cbl) + p, 512*nh + j]
    wproj_h = np.ascontiguousarray(
        proj_w.reshape(2, 4, 128, 2, 512).transpose(3, 0, 2, 1, 4)
        .reshape(4, 128, 4 * 512)).astype(bf)
    # wfc[htg][p, 512*cb + 128*sub + j] = fw[128*cb + p, 512*htg + 128*sub + j]
    wfc_h = np.ascontiguousarray(
        fw.reshape(CB, 128, 8, 512).transpose(2, 1, 0, 3).reshape(8, 128, 8 * 512)
    ).astype(bf)
    bfc_h = np.ascontiguousarray(fb.reshape(HT, 128).T).astype(f32)
    wfcp_h = np.ascontiguousarray(fc_proj_w.reshape(HT, 128, 1024)).astype(bf)

    maskd = np.zeros((4, 128, 512), np.float32)
    for m in range(4):
        maskd[m] = (128 * m + np.arange(128)[:, None]) <= np.arange(512)[None, :]
    maskd = maskd.astype(bf)
    ident = np.eye(128, dtype=np.float32).astype(bf)

    shared = dict(
        xT8=xT8,
        wproj=wproj_h, bproj=proj_b.reshape(1, C).astype(f32),
        wfc=wfc_h, bfc=bfc_h,
        wfcp=wfcp_h, bfcp=fc_proj_b.reshape(1, C).astype(f32),
        maskd=maskd, ident=ident, onesr=np.ones((1, 128), f32),
    )

    in_maps = []
    for c in range(NCORES):
        h0 = 2 * c
        qcols = aw[:, 64 * h0:64 * h0 + 128] * sc          # [1024, 128] both heads' q
        kcols = aw[:, C + 64 * h0:C + 64 * h0 + 128]
        vcols = aw[:, 2 * C + 64 * h0:2 * C + 64 * h0 + 128]
        wqk_full = np.concatenate([qcols, kcols], axis=1).astype(bf)
        wqk_c = wqk_full.reshape(CB, 128, 256)
        bq = ab[64 * h0:64 * h0 + 128] * sc
        bk = ab[C + 64 * h0:C + 64 * h0 + 128]
        bqk_c = np.concatenate([bq, bk]).reshape(256, 1).astype(f32)
        cqk_c = wqk_full.astype(f32).sum(axis=0).reshape(256, 1).astype(f32)
        wv_full = vcols.astype(bf)
        wv_c = wv_full.reshape(CB, 128, 128)
        bv_c = ab[2 * C + 64 * h0:2 * C + 64 * h0 + 128].reshape(128, 1).astype(f32)
        cv_c = wv_full.astype(f32).sum(axis=0).reshape(128, 1).astype(f32)
        xrows_c = xg[RPC * c:RPC * (c + 1)].reshape(4, 128, C).astype(f32)
        m = dict(shared)
        m.update(wqk=wqk_c, bqk=bqk_c, cqk=cqk_c, wv=wv_c, bv=bv_c, cv=cv_c,
                 xrows=xrows_c)
        in_maps.append(m)
    return in_maps


def kernel(**inputs) -> np.ndarray:
    if "nc" not in _cache:
        _cache["nc"] = build()
    nc = _cache["nc"]
    in_maps = _prep(inputs)
    res = bass_utils.run_bass_kernel_spmd(nc, in_maps, core_ids=list(range(NCORES)))
    out = np.concatenate(
        [res.results[c]["out_rows"].reshape(RPC, C) for c in range(NCORES)], axis=0)
    return out.reshape(B, T, C).astype(np.float32)
